# revision 23
# baseline (speedup 1.0000x reference)
"""BinLinear Trainium2 kernel: out = x @ sign(W)^T + sign(bias).

Full shapes: x [8192, 4096] f32, W [4096, 4096] f32, bias [4096] f32,
out [8192, 4096] f32.

Strategy (8 NeuronCores, data-parallel on the token dim M):
  - Each core gets x_shard = x[1024*i : 1024*(i+1)], full W, full bias and
    computes its out shard [1024, 4096]. No collectives; host concatenates.

  - The swizzle-DMA trigger cost on the issuing engine scales with its
    descriptor-run count (~0.47ns per 128B run + ~0.4us fixed), so W's
    64MB of 128B runs is split across TWO HWDGE issuers: W tiles
    alternate between ScalarE (ACT) and SP. ACT-issued tiles use the
    v1-proven claim+hop discipline; SP cannot claim (no data-dep-capable
    cheap op), so SP-issued DMAs rely on the post-scheduling wait
    COMPRESSOR, which walks the scheduled order reconstructing per-sem
    producer-snapshot clocks and drops any wait transitively implied by a
    kept wait (happens-before soundness; build fails if an instruction
    cannot be reduced to walrus's single sync-wait slot).

  - x^T resident as bf16: even k-tiles stream through a gpsimd SWDGE
    cast-load slot CHAIN (fresh DMA targets, zero recycling waits; DVE
    32x32 stream-transpose writes the resident slot); odd k-tiles load as
    f32 on SP through the shared staging pool, DVE-transpose, and
    downcast-copy. Splitting x across two DGE paths roughly halves the
    x-phase critical path that throttled strip 0/1.

  - W streams once as PAIR tiles [L*512 n, 128 k] spanning <=2 n-strips,
    STAGGERED by kt parity so every strip re-produces only half the tiles
    (uniform ~8MB/strip DMA demand). Per tile: 4 swizzled f32 DMAs ->
    3 DVE touches -> DVE bit-trick sign reading the f32 staging's u16
    HIGH halves ((h & 0x8000) | 0x3f80 == +-1.0 bf16 exactly, half the
    input bytes of an f32 ALU op; it carries the 4th DMA-lane wait) ->
    DVE bf16 transpose into kt's resident W^T slot (bufs=1; slot-WAR
    [PE] wait on the old tile's last matmul).

  - TensorE accumulates psum[mi] (8 banks) over 32 k-tiles; rhs is a
    512-col slice of the pair tile. sign(bias) (+-1, same u16 trick)
    enters via a rank-1 ones matmul. PSUM holds the exact output;
    eviction is a plain full-bank DVE copy and the out-DMA (gpsimd)
    follows with its natural RAW wait (own-lane wait compressed away
    through the eviction's slot-WAR chain, outp bufs=8 pinning each slot
    to one DMASW lane).
"""

import numpy as np

import concourse.bass as bass
import concourse.mybir as mybir
import concourse.tile as tile
from concourse.vector_clock import ScopedClock, VectorClock
from concourse.tile import add_dep_helper
from concourse.bass_utils import run_bass_kernel_spmd


class SplitDrainTileContext(tile.TileContext):
    """TileContext whose kernel-tail drain is split into several drain
    instructions. The stock tail emits ONE drain waiting on every active proc
    (engines + all DMA lanes, ~15 waits) which overflows the CTRL
    instruction's sync-wait slots in walrus codegen. Emitting the same waits
    across several drains (1 wait each) is semantically identical."""

    MAX_DRAIN_WAITS = 1

    def _drain_and_barrier(self, tick_clock, wait_clock):
        gc = tick_clock.global_clock
        n = len(gc)
        for lo in range(0, n, self.MAX_DRAIN_WAITS):
            vc = VectorClock()
            for p in range(lo, min(lo + self.MAX_DRAIN_WAITS, n)):
                if gc[p]:
                    vc.require_at_least(p, gc[p])
            drain_inst = self.nc.sync.drain()
            wait_clock.add_sem_waits(
                drain_inst.ins, ScopedClock({None: vc})
            )
        self.nc.all_engine_barrier()
        assert self.sems is not None
        popped = self.nc._tile_sem_poison_stack.pop()
        assert popped is self._sem_poison
        self.nc.clear_and_free_semaphores(list(self.sems.allocated().values()))
        self.nc.all_engine_barrier()


P = 128
NFREE = 512  # moving free dim per matmul (one PSUM bank of fp32)

M_FULL, K_FULL, N_FULL = 8192, 4096, 4096
N_CORES = 8
M_SHARD = M_FULL // N_CORES


def _swizzled_load(nc, sbuf_tile, dram_ap, eng):
    """Load dram_ap ([R, 128] slice) into sbuf_tile [128, R] block-swizzled so
    that a DVE 32x32 stream-transpose of sbuf_tile yields dram_ap.T.

    Pre-DVE we need:  sbuf[32g+a, 32b+c] = dram[32b+a, 32g+c]
    so post-DVE:      out[32g+a, 32b+c] = dram[32b+c, 32g+a] = dram.T[p, f].

    DMA access patterns are limited to 3 dims, so issue one DMA per
    partition-group g (source dims [a, b, c], 128-byte contiguous runs).
    A dtype mismatch (f32 dram -> bf16 sbuf) is legal only on the SWDGE
    (gpsimd) path, which casts during the DMA.
    """
    for g in range(4):
        eng.dma_start(
            sbuf_tile[32 * g : 32 * (g + 1), :],
            dram_ap[:, 32 * g : 32 * (g + 1)].rearrange("(b a) c -> a b c", a=32),
        )


def _touch3(nc, sbuf_tile):
    """In-place 1-element DVE copies for partition groups 0-2, each waiting
    on one swizzle DMA's lane; the full-width DVE consumer that follows
    carries group 3's lane wait itself (its single allowed wait)."""
    for g in range(3):
        s = sbuf_tile[32 * g : 32 * (g + 1), 0:1]
        nc.vector.tensor_copy(out=s, in_=s)


# W tile coverage, staggered by kt parity: list of (first_strip, n_strips).
_W_SPANS_EVEN = [(0, 2), (2, 2), (4, 2), (6, 2)]
_W_SPANS_ODD = [(0, 1), (1, 2), (3, 2), (5, 2), (7, 1)]


def _w_spans(kt):
    return _W_SPANS_EVEN if kt % 2 == 0 else _W_SPANS_ODD


def bin_linear_tile_kernel(tc, x_ap, w_ap, b_ap, o_ap):
    nc = tc.nc
    f32 = mybir.dt.float32
    bf16 = mybir.dt.bfloat16
    u16 = mybir.dt.uint16

    MS, K = x_ap.shape  # m per core, contraction
    N = w_ap.shape[0]
    KT = K // P  # k tiles
    KTH = KT // 2
    MT = MS // P  # m tiles (psum banks used per n-strip)
    NS = N // NFREE  # n strips
    assert MT <= 8, "psum accumulators exceed the 8 PSUM banks"
    assert NS == 8 and KT % 2 == 0, "stagger tables assume 8 strips, even KT"

    with (
        tc.tile_pool(name="xe", bufs=1) as xe_pool,
        tc.tile_pool(name="xo", bufs=1) as xo_pool,
        tc.tile_pool(name="xtr", bufs=2) as xtr_pool,
        tc.tile_pool(name="wstg", bufs=2) as wstg_pool,
        tc.tile_pool(name="wsgn", bufs=2) as wsgn_pool,
        tc.tile_pool(name="wt", bufs=1) as wt_pool,
        tc.tile_pool(name="outp", bufs=8) as out_pool,
        tc.tile_pool(name="bias", bufs=1) as bias_pool,
        tc.tile_pool(name="psum", bufs=8, space="PSUM") as psum_pool,
    ):
        # ---- bias + ones first so strip-0's bias matmuls head the queues.
        # sign(bias) [1, N] bf16 (+-1 exact): SWDGE cast-load f32->bf16,
        # then one in-place DVE bitwise op: (b & 0x8000) | 0x3f80.
        bias_sgn = bias_pool.tile([1, N], bf16)
        ones_row = bias_pool.tile([1, P], bf16)
        pscr = bias_pool.tile([1, 1], f32, name="pscr", tag="pscr", bufs=2)
        nc.gpsimd.dma_start(bias_sgn[:], b_ap[None, :])
        bsu = bias_sgn[:].bitcast(u16)
        nc.vector.tensor_scalar(
            out=bsu,
            in0=bsu,
            scalar1=0x8000,
            scalar2=0x3F80,
            op0=mybir.AluOpType.bitwise_and,
            op1=mybir.AluOpType.bitwise_or,
        )
        nc.scalar.activation(
            ones_row[:],
            bias_sgn[:, 0:P],
            mybir.ActivationFunctionType.Copy,
            bias=1.0,
            scale=0.0,
        )

        # Staging pool depth 2: allocation j recycles the slot of j-2,
        # whose last DVE reader postdates the touches of j-2 -- and j-2
        # shares this tile's 4 DMAHW lanes (4 DMAs/tile over 8 lanes), so
        # the single kept [DVE >= reader(j-2)] wait on the first DMA
        # transitively dominates the own-lane wait. No ACT claims needed.
        STG_BUFS = 2

        # ---- x^T resident, split by kt parity -------------------------
        # Even kt (gpsimd cast-load slot chain): one [128, (KTH+1)*MS] bf16
        # tile; the cast DMA for even-tile e fills chain slot e+1, the DVE
        # transpose writes slot e = resident x^T. Fresh DMA targets => the
        # casts carry only their own DMASW-lane wait.
        xe = xe_pool.tile([P, (KTH + 1) * MS], bf16, name="xe")
        # Odd kt (SP f32 via the shared staging pool): transpose -> xtr,
        # downcast-copy into xo.
        xo = xo_pool.tile([P, KTH, MS], bf16, name="xo")

        def xt_slice(kt, lo, hi):
            if kt % 2 == 0:
                e = kt // 2
                return xe[:, e * MS + lo : e * MS + hi]
            return xo[:, kt // 2, lo:hi]

        for kt in range(KT):
            if kt % 2 == 0:
                e = kt // 2
                stg = xe[:, (e + 1) * MS : (e + 2) * MS]
                _swizzled_load(nc, stg, x_ap[:, kt * P : (kt + 1) * P],
                               nc.gpsimd)
                _touch3(nc, stg)
                nc.vector.transpose(xe[:, e * MS : (e + 1) * MS], stg)
            else:
                ws = wstg_pool.tile([P, MS], f32, name=f"xs_{kt}", tag="ws",
                                    bufs=STG_BUFS)
                _swizzled_load(nc, ws, x_ap[:, kt * P : (kt + 1) * P],
                               nc.sync)
                _touch3(nc, ws)
                xtr = xtr_pool.tile([P, MS], f32, name=f"xtr_{kt}", tag="xtr",
                                    bufs=2)
                # carries the 4th DMA lane wait as its single cross wait
                nc.vector.transpose(xtr[:], ws[:])
                nc.vector.tensor_copy(out=xo[:, kt // 2, :], in_=xtr[:])

        # ---- W tile pipeline ------------------------------------------
        n_w = [0]

        def emit_w_tile(kt, span):
            s0, L = span
            R = L * NFREE
            on_act = n_w[0] % 2 == 0
            n_w[0] += 1
            ws = wstg_pool.tile([P, R], f32, name=f"ws_{kt}_{s0}", tag="ws",
                                bufs=STG_BUFS)
            n0 = s0 * NFREE
            _swizzled_load(nc, ws, w_ap[n0 : n0 + R, kt * P : (kt + 1) * P],
                           nc.scalar if on_act else nc.sync)
            _touch3(nc, ws)
            wg = wsgn_pool.tile([P, R], bf16, name=f"wg_{kt}_{s0}", tag="wg",
                                bufs=2)
            # sign via the f32 words' u16 HIGH halves: (h & 0x8000) | 0x3f80
            # is exactly +-1.0 bf16; reads 2 bytes/elem instead of 4 and
            # carries the 4th DMA lane wait as its single cross wait.
            hi = ws[:].bitcast(u16)[:, 1::2]
            nc.vector.tensor_scalar(
                out=wg[:].bitcast(u16),
                in0=hi,
                scalar1=0x8000,
                scalar2=0x3F80,
                op0=mybir.AluOpType.bitwise_and,
                op1=mybir.AluOpType.bitwise_or,
            )
            wt = wt_pool.tile([P, R], bf16, name=f"wt_{kt}_{s0}",
                              tag=f"wt{kt}", bufs=1)
            nc.vector.transpose(wt[:], wg[:])
            return (wt, s0)

        wt_cur = [emit_w_tile(kt, _w_spans(kt)[0]) for kt in range(KT)]

        def next_span(kt, ns):
            for sp in _w_spans(kt):
                if sp[0] == ns + 1:
                    return sp
            return None

        # ---- PSUM accumulators allocated ONCE (no per-strip realloc waits).
        psums = [
            psum_pool.tile([P, NFREE], f32, name=f"psum_{mi}", tag="acc")
            for mi in range(MT)
        ]

        for ns in range(NS):
            n_lo = ns * NFREE
            # bias enters PSUM first: rank-1 matmul, start=True clears banks.
            for mi in range(MT):
                nc.tensor.matmul(
                    psums[mi][:],
                    ones_row[:],
                    bias_sgn[:, n_lo : n_lo + NFREE],
                    start=True,
                    stop=False,
                )
            for kt in range(KT):
                wt, s0 = wt_cur[kt]
                half = (ns - s0) * NFREE
                rhs = wt[:, half : half + NFREE]
                last = kt == KT - 1
                for mi in range(MT):
                    nc.tensor.matmul(
                        psums[mi][:],
                        xt_slice(kt, mi * P, (mi + 1) * P),
                        rhs,
                        start=False,
                        stop=last,
                    )
                # Refill kt's W slot for the span starting at strip ns+1
                # (emitted after this kt's matmuls so the slot-WAR transpose
                # gates mid-strip; DMA demand is ~16 tiles every strip).
                sp = next_span(kt, ns)
                if sp is not None:
                    wt_cur[kt] = emit_w_tile(kt, sp)

            # A read-only 1-elem DVE copy of the LAST bank: it waits for the
            # final stop-matmul of the strip, putting PE on DVE's clock so
            # every eviction copy below elides its PE wait. (Read-only so
            # the mi=7 evict has no in-pipeline WAR against it.)
            pe_touch = nc.vector.tensor_copy(
                out=pscr[0:1, 0:1], in_=psums[MT - 1][0:1, 0:1]
            )
            # Evict full banks with plain DVE copies (PSUM already holds the
            # exact output); each out-DMA follows with its natural RAW wait
            # (the compressor drops the own-lane wait: bufs=8 pins each out
            # slot to one DMASW lane, so the eviction's slot-WAR wait on the
            # previous strip's out-DMA covers exactly that lane).
            for mi in range(MT):
                ot = out_pool.tile(
                    [P, NFREE], f32, name=f"ot_{ns}_{mi}", tag="ot", bufs=8
                )
                cpi = nc.vector.tensor_copy(out=ot[:], in_=psums[mi][:])
                add_dep_helper(cpi.ins, pe_touch.ins, sync=False,
                               reason="evac copy after PE-observing touch")
                nc.gpsimd.dma_start(
                    o_ap[mi * P : (mi + 1) * P, n_lo : n_lo + NFREE],
                    ot[:],
                )


# Engines whose own-proc-sem waits at past positions are droppable: they are
# single-threaded and retire data effects in queue order. (Pool = 8 Q7 cores
# running concurrently; PE reorders LDWEIGHTS: keep theirs.)
_OWN_DROP_ENGINES = {
    "EngineType.DVE": "DVE",
    "EngineType.Activation": "Activation",
    "EngineType.SP": "SP",
}


def _compress_waits(ordered_insts):
    """Post-scheduling wait compression: drop waits that are transitively
    implied (happens-before) by another wait on the same instruction, so
    every instruction fits walrus's one-sync-wait slot.

    Walking the scheduled order we maintain, per issuing engine, the
    observed clock (join of all waits executed so far plus the producer
    snapshots those waits import), and record for every semaphore update
    (sem, cumulative-value) the producer's knowledge at that point. A wait
    w on instruction X is droppable iff some kept wait (S >= v) on X has
    snapshot(S, v)[w.sem] >= w.value: the kept wait then transitively
    enforces w before X runs, and queue order preserves it for every later
    instruction whose emitted waits assumed X's. Raises if an instruction
    cannot be reduced to a single wait."""
    snap = {}       # (sem id, value) -> dict(sem id -> value), across passes

    def merge(dst, src):
        for s2, v2 in src.items():
            if dst.get(s2, -1) < v2:
                dst[s2] = v2

    def walk(compress):
        # The block list interleaves engines in a non-temporal order; only
        # per-engine subsequences are queue-ordered. Snapshots from earlier
        # passes resolve forward references, converging monotonically.
        cum = {}    # sem id -> cumulative value
        obs = {}    # engine -> dict(sem id -> value)
        failures = []
        for ins in ordered_insts:
            si = ins.sync_info
            if si is None:
                continue
            know = obs.setdefault(ins.engine, {})
            waits = list(si.on_wait)
            know_before = dict(know)
            if waits:
                for w in waits:
                    if w.wait_value is None:
                        continue
                    merge(know, {w.id: w.wait_value})
                    merge(know, snap.get((w.id, w.wait_value), {}))
                if compress and len(waits) > 1:
                    # a wait already implied by the engine's reconstructed
                    # observed clock (its own earlier waits + their
                    # snapshots) is redundant outright; so is a wait on the
                    # engine's OWN proc sem at a past position (in-order
                    # single-threaded engines -- DVE/ACT/SP -- retire
                    # effects in queue order; such waits appear only as
                    # bookkeeping artifacts after TensorScalarPtr ops,
                    # which skip Tile's same-engine elision). Pool is
                    # genuinely multi-core and PE reorders LDWEIGHTS, so
                    # their own-sem waits are kept.
                    own = _OWN_DROP_ENGINES.get(str(ins.engine))
                    needed = [
                        w for w in waits
                        if w.wait_value is not None
                        and know_before.get(w.id, -1) < w.wait_value
                        and not (
                            own is not None
                            and (w.ant_name or "").startswith(own)
                            and cum.get(w.id, 0) >= w.wait_value
                        )
                    ]
                    keeper = needed[0] if len(needed) == 1 else None
                    if keeper is None and len(needed) > 1:
                        for w in needed:
                            ks = dict(snap.get((w.id, w.wait_value), {}))
                            merge(ks, {w.id: w.wait_value})
                            if all(
                                w2 is w
                                or ks.get(w2.id, -1) >= w2.wait_value
                                for w2 in needed
                            ):
                                keeper = w
                                break
                        if keeper is None:
                            failures.append(
                                (ins.name, type(ins).__name__,
                                 str(ins.engine),
                                 [(w.ant_name, w.wait_value) for w in waits])
                            )
                    if keeper is not None:
                        si.on_wait[:] = [keeper]
                    elif not needed:
                        si.on_wait[:] = waits[:1]
            for u in si.on_update:
                if u.update_mode == "sem-inc":
                    inc = 1 if u.update_value is None else u.update_value
                elif u.update_mode == "sem-add-imm" and u.update_value is not None:
                    inc = u.update_value
                else:
                    cum.pop(u.id, None)
                    continue
                nv = cum.get(u.id, 0) + inc
                cum[u.id] = nv
                merged = snap.setdefault((u.id, nv), {})
                merge(merged, know)
        return failures

    walk(False)
    walk(False)
    failures = walk(True)
    if failures:
        raise RuntimeError(
            f"wait compression failed for {len(failures)} instructions: "
            + "; ".join(str(f) for f in failures[:8])
        )


def build_module(m_shard=M_SHARD, k=K_FULL, n=N_FULL):
    nc = bass.Bass("TRN2", target_bir_lowering=False, debug=False)
    f32 = mybir.dt.float32
    x_d = nc.dram_tensor("x", [m_shard, k], f32, kind="ExternalInput")
    w_d = nc.dram_tensor("weight", [n, k], f32, kind="ExternalInput")
    b_d = nc.dram_tensor("bias", [n], f32, kind="ExternalInput")
    o_d = nc.dram_tensor("out", [m_shard, n], f32, kind="ExternalOutput")
    with SplitDrainTileContext(nc) as tc:
        bin_linear_tile_kernel(tc, x_d.ap(), w_d.ap(), b_d.ap(), o_d.ap())
    ordered = []
    for bb, insts in tc.ordered_instructions_by_block.items():
        ordered.extend(insts)
    _compress_waits(ordered)
    return nc


_NC_CACHE = {}


def _get_module():
    if "nc" not in _NC_CACHE:
        _NC_CACHE["nc"] = build_module()
    return _NC_CACHE["nc"]


def make_in_maps(x, weight, bias):
    x = np.ascontiguousarray(np.asarray(x, dtype=np.float32))
    weight = np.ascontiguousarray(np.asarray(weight, dtype=np.float32))
    bias = np.ascontiguousarray(np.asarray(bias, dtype=np.float32))
    return [
        {
            "x": x[i * M_SHARD : (i + 1) * M_SHARD],
            "weight": weight,
            "bias": bias,
        }
        for i in range(N_CORES)
    ]


def gather(results):
    return np.concatenate([results[i]["out"] for i in range(N_CORES)], axis=0)


def run(x, weight, bias, trace=False, **kw):
    """Run on the 8 NeuronCores; returns (out_full, BassKernelResults)."""
    nc = _get_module()
    in_maps = make_in_maps(x, weight, bias)
    res = run_bass_kernel_spmd(nc, in_maps, list(range(N_CORES)), trace=trace, **kw)
    return gather(res.results), res


def kernel(x, weight, bias):
    out, _ = run(x, weight, bias)
    return out


# revision 25
# speedup vs baseline: 1.0879x; 1.0879x over previous
"""BinLinear Trainium2 kernel: out = x @ sign(W)^T + sign(bias).

Full shapes: x [8192, 4096] f32, W [4096, 4096] f32, bias [4096] f32,
out [8192, 4096] f32.

Strategy (8 NeuronCores, data-parallel on the token dim M):
  - Each core gets x_shard = x[1024*i : 1024*(i+1)], full W, full bias and
    computes its out shard [1024, 4096]. No collectives; host concatenates.

  - The swizzle-DMA trigger cost on the issuing engine scales with its
    descriptor-run count (~0.47ns per 128B run + ~0.4us fixed), so W's
    64MB of 128B runs is split across TWO HWDGE issuers: W tiles
    alternate between ScalarE (ACT) and SP. ACT-issued tiles use the
    v1-proven claim+hop discipline; SP cannot claim (no data-dep-capable
    cheap op), so SP-issued DMAs rely on the post-scheduling wait
    COMPRESSOR, which walks the scheduled order reconstructing per-sem
    producer-snapshot clocks and drops any wait transitively implied by a
    kept wait (happens-before soundness; build fails if an instruction
    cannot be reduced to walrus's single sync-wait slot).

  - x^T resident as bf16: even k-tiles stream through a gpsimd SWDGE
    cast-load slot CHAIN (fresh DMA targets, zero recycling waits; DVE
    32x32 stream-transpose writes the resident slot); odd k-tiles load as
    f32 on SP through the shared staging pool, DVE-transpose, and
    downcast-copy. Splitting x across two DGE paths roughly halves the
    x-phase critical path that throttled strip 0/1.

  - W streams once as PAIR tiles [L*512 n, 128 k] spanning <=2 n-strips,
    STAGGERED by kt parity so every strip re-produces only half the tiles
    (uniform ~8MB/strip DMA demand). Per tile: 4 swizzled f32 DMAs ->
    3 DVE touches -> DVE bit-trick sign reading the f32 staging's u16
    HIGH halves ((h & 0x8000) | 0x3f80 == +-1.0 bf16 exactly, half the
    input bytes of an f32 ALU op; it carries the 4th DMA-lane wait) ->
    DVE bf16 transpose into kt's resident W^T slot (bufs=1; slot-WAR
    [PE] wait on the old tile's last matmul).

  - TensorE accumulates psum[mi] (8 banks) over 32 k-tiles; rhs is a
    512-col slice of the pair tile. sign(bias) (+-1, same u16 trick)
    enters via a rank-1 ones matmul. PSUM holds the exact output;
    eviction is a plain full-bank DVE copy and the out-DMA (gpsimd)
    follows with its natural RAW wait (own-lane wait compressed away
    through the eviction's slot-WAR chain, outp bufs=8 pinning each slot
    to one DMASW lane).
"""

import numpy as np

import concourse.bass as bass
import concourse.mybir as mybir
import concourse.tile as tile
from concourse.vector_clock import ScopedClock, VectorClock
from concourse.tile import add_dep_helper
from concourse.bass_utils import run_bass_kernel_spmd


class SplitDrainTileContext(tile.TileContext):
    """TileContext whose kernel-tail drain is split into several drain
    instructions. The stock tail emits ONE drain waiting on every active proc
    (engines + all DMA lanes, ~15 waits) which overflows the CTRL
    instruction's sync-wait slots in walrus codegen. Emitting the same waits
    across several drains (1 wait each) is semantically identical."""

    MAX_DRAIN_WAITS = 1

    def _drain_and_barrier(self, tick_clock, wait_clock):
        gc = tick_clock.global_clock
        n = len(gc)
        for lo in range(0, n, self.MAX_DRAIN_WAITS):
            vc = VectorClock()
            for p in range(lo, min(lo + self.MAX_DRAIN_WAITS, n)):
                if gc[p]:
                    vc.require_at_least(p, gc[p])
            drain_inst = self.nc.sync.drain()
            wait_clock.add_sem_waits(
                drain_inst.ins, ScopedClock({None: vc})
            )
        self.nc.all_engine_barrier()
        assert self.sems is not None
        popped = self.nc._tile_sem_poison_stack.pop()
        assert popped is self._sem_poison
        self.nc.clear_and_free_semaphores(list(self.sems.allocated().values()))
        self.nc.all_engine_barrier()


P = 128
NFREE = 512  # moving free dim per matmul (one PSUM bank of fp32)

M_FULL, K_FULL, N_FULL = 8192, 4096, 4096
N_CORES = 8
M_SHARD = M_FULL // N_CORES


def _swizzled_load(nc, sbuf_tile, dram_ap, eng):
    """Load dram_ap ([R, 128] slice) into sbuf_tile [128, R] block-swizzled so
    that a DVE 32x32 stream-transpose of sbuf_tile yields dram_ap.T.

    Pre-DVE we need:  sbuf[32g+a, 32b+c] = dram[32b+a, 32g+c]
    so post-DVE:      out[32g+a, 32b+c] = dram[32b+c, 32g+a] = dram.T[p, f].

    DMA access patterns are limited to 3 dims, so issue one DMA per
    partition-group g (source dims [a, b, c], 128-byte contiguous runs).
    A dtype mismatch (f32 dram -> bf16 sbuf) is legal only on the SWDGE
    (gpsimd) path, which casts during the DMA.
    """
    for g in range(4):
        eng.dma_start(
            sbuf_tile[32 * g : 32 * (g + 1), :],
            dram_ap[:, 32 * g : 32 * (g + 1)].rearrange("(b a) c -> a b c", a=32),
        )


def _touch3(nc, sbuf_tile):
    """In-place 1-element DVE copies for partition groups 0-2, each waiting
    on one swizzle DMA's lane; the full-width DVE consumer that follows
    carries group 3's lane wait itself (its single allowed wait)."""
    for g in range(3):
        s = sbuf_tile[32 * g : 32 * (g + 1), 0:1]
        nc.vector.tensor_copy(out=s, in_=s)


# W tile coverage, staggered by kt parity: list of (first_strip, n_strips).
_W_SPANS_EVEN = [(0, 2), (2, 2), (4, 2), (6, 2)]
_W_SPANS_ODD = [(0, 1), (1, 2), (3, 2), (5, 2), (7, 1)]


def _w_spans(kt):
    return _W_SPANS_EVEN if kt % 2 == 0 else _W_SPANS_ODD


def bin_linear_tile_kernel(tc, x_ap, w_ap, b_ap, o_ap):
    nc = tc.nc
    f32 = mybir.dt.float32
    bf16 = mybir.dt.bfloat16
    u16 = mybir.dt.uint16

    MS, K = x_ap.shape  # m per core, contraction
    N = w_ap.shape[0]
    KT = K // P  # k tiles
    KTH = KT // 2
    MT = MS // P  # m tiles (psum banks used per n-strip)
    NS = N // NFREE  # n strips
    assert MT <= 8, "psum accumulators exceed the 8 PSUM banks"
    assert NS == 8 and KT % 2 == 0, "stagger tables assume 8 strips, even KT"

    with (
        tc.tile_pool(name="xe", bufs=1) as xe_pool,
        tc.tile_pool(name="wstg", bufs=6) as wstg_pool,
        tc.tile_pool(name="wsgn", bufs=2) as wsgn_pool,
        tc.tile_pool(name="wt", bufs=1) as wt_pool,
        tc.tile_pool(name="outp", bufs=8) as out_pool,
        tc.tile_pool(name="bias", bufs=1) as bias_pool,
        tc.tile_pool(name="psum", bufs=8, space="PSUM") as psum_pool,
    ):
        # ---- bias + ones first so strip-0's bias matmuls head the queues.
        # sign(bias) [1, N] bf16 (+-1 exact): SWDGE cast-load f32->bf16,
        # then one in-place DVE bitwise op: (b & 0x8000) | 0x3f80.
        bias_sgn = bias_pool.tile([1, N], bf16)
        ones_row = bias_pool.tile([1, P], bf16)
        pscr = bias_pool.tile([1, 1], f32, name="pscr", tag="pscr", bufs=2)
        ascr = bias_pool.tile([1, 1], f32, name="ascr", tag="ascr", bufs=2)
        nc.gpsimd.dma_start(bias_sgn[:], b_ap[None, :])
        bsu = bias_sgn[:].bitcast(u16)
        nc.vector.tensor_scalar(
            out=bsu,
            in0=bsu,
            scalar1=0x8000,
            scalar2=0x3F80,
            op0=mybir.AluOpType.bitwise_and,
            op1=mybir.AluOpType.bitwise_or,
        )
        nc.scalar.activation(
            ones_row[:],
            bias_sgn[:, 0:P],
            mybir.ActivationFunctionType.Copy,
            bias=1.0,
            scale=0.0,
        )

        # W staging (ACT, bufs=6) uses the v1-proven 1-elem ACT claim +
        # DVE hop, reading the W^T retire beacon from 6 tiles back.
        W_STG_BUFS = 6
        wt_beacons = []

        # ---- x^T resident (gpsimd cast-load slot chain) ---------------
        # One [128, (KT+1)*MS] bf16 tile; the cast DMA for k-tile kt fills
        # chain slot kt+1, the DVE 32x32 stream-transpose writes slot kt =
        # the resident x^T tile. Fresh DMA targets => the casts carry only
        # their own DMASW-lane wait; the LDWEIGHTS' inherited staging-lane
        # waits are removed by the compression pass.
        xe = xe_pool.tile([P, (KT + 1) * MS], bf16, name="xe")

        def xt_slice(kt, lo, hi):
            return xe[:, kt * MS + lo : kt * MS + hi]

        for kt in range(KT):
            stg = xe[:, (kt + 1) * MS : (kt + 2) * MS]
            _swizzled_load(nc, stg, x_ap[:, kt * P : (kt + 1) * P],
                           nc.gpsimd)
            _touch3(nc, stg)
            nc.vector.transpose(xe[:, kt * MS : (kt + 1) * MS], stg)

        # ---- W tile pipeline ------------------------------------------

        def emit_w_tile(kt, span):
            s0, L = span
            R = L * NFREE
            j = len(wt_beacons)
            if j >= W_STG_BUFS:
                h = ascr[0:1, 0:1]
                nc.scalar.activation(
                    h, wt_beacons[j - W_STG_BUFS],
                    mybir.ActivationFunctionType.Copy,
                )
                nc.vector.tensor_copy(out=h, in_=h)
            ws = wstg_pool.tile([P, R], f32, name=f"ws_{kt}_{s0}", tag="ws",
                                bufs=W_STG_BUFS)
            n0 = s0 * NFREE
            _swizzled_load(nc, ws, w_ap[n0 : n0 + R, kt * P : (kt + 1) * P],
                           nc.scalar)
            _touch3(nc, ws)
            wg = wsgn_pool.tile([P, R], bf16, name=f"wg_{kt}_{s0}", tag="wg",
                                bufs=2)
            # sign via the f32 words' u16 HIGH halves: (h & 0x8000) | 0x3f80
            # is exactly +-1.0 bf16; reads 2 bytes/elem instead of 4 and
            # carries the 4th DMA lane wait as its single cross wait.
            hi = ws[:].bitcast(u16)[:, 1::2]
            nc.vector.tensor_scalar(
                out=wg[:].bitcast(u16),
                in0=hi,
                scalar1=0x8000,
                scalar2=0x3F80,
                op0=mybir.AluOpType.bitwise_and,
                op1=mybir.AluOpType.bitwise_or,
            )
            wt = wt_pool.tile([P, R], bf16, name=f"wt_{kt}_{s0}",
                              tag=f"wt{kt}", bufs=1)
            nc.vector.transpose(wt[:], wg[:])
            wt_beacons.append(wt[0:1, 0:1])
            return (wt, s0)

        wt_cur = [emit_w_tile(kt, _w_spans(kt)[0]) for kt in range(KT)]

        def next_span(kt, ns):
            for sp in _w_spans(kt):
                if sp[0] == ns + 1:
                    return sp
            return None

        # ---- PSUM accumulators allocated ONCE (no per-strip realloc waits).
        psums = [
            psum_pool.tile([P, NFREE], f32, name=f"psum_{mi}", tag="acc")
            for mi in range(MT)
        ]

        for ns in range(NS):
            n_lo = ns * NFREE
            # bias enters PSUM first: rank-1 matmul, start=True clears banks.
            for mi in range(MT):
                nc.tensor.matmul(
                    psums[mi][:],
                    ones_row[:],
                    bias_sgn[:, n_lo : n_lo + NFREE],
                    start=True,
                    stop=False,
                )
            for kt in range(KT):
                wt, s0 = wt_cur[kt]
                half = (ns - s0) * NFREE
                rhs = wt[:, half : half + NFREE]
                last = kt == KT - 1
                for mi in range(MT):
                    nc.tensor.matmul(
                        psums[mi][:],
                        xt_slice(kt, mi * P, (mi + 1) * P),
                        rhs,
                        start=False,
                        stop=last,
                    )
                # Refill kt's W slot for the span starting at strip ns+1
                # (emitted after this kt's matmuls so the slot-WAR transpose
                # gates mid-strip; DMA demand is ~16 tiles every strip).
                sp = next_span(kt, ns)
                if sp is not None:
                    wt_cur[kt] = emit_w_tile(kt, sp)

            # A read-only 1-elem DVE copy of the LAST bank: it waits for the
            # final stop-matmul of the strip, putting PE on DVE's clock so
            # every eviction copy below elides its PE wait. (Read-only so
            # the mi=7 evict has no in-pipeline WAR against it.)
            pe_touch = nc.vector.tensor_copy(
                out=pscr[0:1, 0:1], in_=psums[MT - 1][0:1, 0:1]
            )
            # Evict full banks with plain DVE copies (PSUM already holds the
            # exact output); each out-DMA follows with its natural RAW wait
            # (the compressor drops the own-lane wait: bufs=8 pins each out
            # slot to one DMASW lane, so the eviction's slot-WAR wait on the
            # previous strip's out-DMA covers exactly that lane).
            for mi in range(MT):
                ot = out_pool.tile(
                    [P, NFREE], f32, name=f"ot_{ns}_{mi}", tag="ot", bufs=8
                )
                cpi = nc.vector.tensor_copy(out=ot[:], in_=psums[mi][:])
                add_dep_helper(cpi.ins, pe_touch.ins, sync=False,
                               reason="evac copy after PE-observing touch")
                nc.gpsimd.dma_start(
                    o_ap[mi * P : (mi + 1) * P, n_lo : n_lo + NFREE],
                    ot[:],
                )


# Engines whose own-proc-sem waits at past positions are droppable: they are
# single-threaded and retire data effects in queue order. (Pool = 8 Q7 cores
# running concurrently; PE reorders LDWEIGHTS: keep theirs.)
_OWN_DROP_ENGINES = {
    "EngineType.DVE": "DVE",
    "EngineType.Activation": "Activation",
    "EngineType.SP": "SP",
}


def _compress_waits(ordered_insts):
    """Post-scheduling wait compression: drop waits that are transitively
    implied (happens-before) by another wait on the same instruction, so
    every instruction fits walrus's one-sync-wait slot.

    Walking the scheduled order we maintain, per issuing engine, the
    observed clock (join of all waits executed so far plus the producer
    snapshots those waits import), and record for every semaphore update
    (sem, cumulative-value) the producer's knowledge at that point. A wait
    w on instruction X is droppable iff some kept wait (S >= v) on X has
    snapshot(S, v)[w.sem] >= w.value: the kept wait then transitively
    enforces w before X runs, and queue order preserves it for every later
    instruction whose emitted waits assumed X's. Raises if an instruction
    cannot be reduced to a single wait."""
    snap = {}       # (sem id, value) -> dict(sem id -> value), across passes

    def merge(dst, src):
        for s2, v2 in src.items():
            if dst.get(s2, -1) < v2:
                dst[s2] = v2

    def walk(compress):
        # The block list interleaves engines in a non-temporal order; only
        # per-engine subsequences are queue-ordered. Snapshots from earlier
        # passes resolve forward references, converging monotonically.
        cum = {}    # sem id -> cumulative value
        obs = {}    # engine -> dict(sem id -> value)
        failures = []
        for ins in ordered_insts:
            si = ins.sync_info
            if si is None:
                continue
            know = obs.setdefault(ins.engine, {})
            waits = list(si.on_wait)
            know_before = dict(know)
            if waits:
                for w in waits:
                    if w.wait_value is None:
                        continue
                    merge(know, {w.id: w.wait_value})
                    merge(know, snap.get((w.id, w.wait_value), {}))
                if compress and len(waits) > 1:
                    # a wait already implied by the engine's reconstructed
                    # observed clock (its own earlier waits + their
                    # snapshots) is redundant outright; so is a wait on the
                    # engine's OWN proc sem at a past position (in-order
                    # single-threaded engines -- DVE/ACT/SP -- retire
                    # effects in queue order; such waits appear only as
                    # bookkeeping artifacts after TensorScalarPtr ops,
                    # which skip Tile's same-engine elision). Pool is
                    # genuinely multi-core and PE reorders LDWEIGHTS, so
                    # their own-sem waits are kept.
                    own = _OWN_DROP_ENGINES.get(str(ins.engine))
                    needed = [
                        w for w in waits
                        if w.wait_value is not None
                        and know_before.get(w.id, -1) < w.wait_value
                        and not (
                            own is not None
                            and (w.ant_name or "").startswith(own)
                            and cum.get(w.id, 0) >= w.wait_value
                        )
                    ]
                    keeper = needed[0] if len(needed) == 1 else None
                    if keeper is None and len(needed) > 1:
                        for w in needed:
                            ks = dict(snap.get((w.id, w.wait_value), {}))
                            merge(ks, {w.id: w.wait_value})
                            if all(
                                w2 is w
                                or ks.get(w2.id, -1) >= w2.wait_value
                                for w2 in needed
                            ):
                                keeper = w
                                break
                        if keeper is None:
                            failures.append(
                                (ins.name, type(ins).__name__,
                                 str(ins.engine),
                                 [(w.ant_name, w.wait_value) for w in waits])
                            )
                    if keeper is not None:
                        si.on_wait[:] = [keeper]
                    elif not needed:
                        si.on_wait[:] = waits[:1]
            for u in si.on_update:
                if u.update_mode == "sem-inc":
                    inc = 1 if u.update_value is None else u.update_value
                elif u.update_mode == "sem-add-imm" and u.update_value is not None:
                    inc = u.update_value
                else:
                    cum.pop(u.id, None)
                    continue
                nv = cum.get(u.id, 0) + inc
                cum[u.id] = nv
                merged = snap.setdefault((u.id, nv), {})
                merge(merged, know)
        return failures

    walk(False)
    walk(False)
    failures = walk(True)
    if failures:
        raise RuntimeError(
            f"wait compression failed for {len(failures)} instructions: "
            + "; ".join(str(f) for f in failures[:8])
        )


def build_module(m_shard=M_SHARD, k=K_FULL, n=N_FULL):
    nc = bass.Bass("TRN2", target_bir_lowering=False, debug=False)
    f32 = mybir.dt.float32
    x_d = nc.dram_tensor("x", [m_shard, k], f32, kind="ExternalInput")
    w_d = nc.dram_tensor("weight", [n, k], f32, kind="ExternalInput")
    b_d = nc.dram_tensor("bias", [n], f32, kind="ExternalInput")
    o_d = nc.dram_tensor("out", [m_shard, n], f32, kind="ExternalOutput")
    with SplitDrainTileContext(nc) as tc:
        bin_linear_tile_kernel(tc, x_d.ap(), w_d.ap(), b_d.ap(), o_d.ap())
    ordered = []
    for bb, insts in tc.ordered_instructions_by_block.items():
        ordered.extend(insts)
    _compress_waits(ordered)
    return nc


_NC_CACHE = {}


def _get_module():
    if "nc" not in _NC_CACHE:
        _NC_CACHE["nc"] = build_module()
    return _NC_CACHE["nc"]


def make_in_maps(x, weight, bias):
    x = np.ascontiguousarray(np.asarray(x, dtype=np.float32))
    weight = np.ascontiguousarray(np.asarray(weight, dtype=np.float32))
    bias = np.ascontiguousarray(np.asarray(bias, dtype=np.float32))
    return [
        {
            "x": x[i * M_SHARD : (i + 1) * M_SHARD],
            "weight": weight,
            "bias": bias,
        }
        for i in range(N_CORES)
    ]


def gather(results):
    return np.concatenate([results[i]["out"] for i in range(N_CORES)], axis=0)


def run(x, weight, bias, trace=False, **kw):
    """Run on the 8 NeuronCores; returns (out_full, BassKernelResults)."""
    nc = _get_module()
    in_maps = make_in_maps(x, weight, bias)
    res = run_bass_kernel_spmd(nc, in_maps, list(range(N_CORES)), trace=trace, **kw)
    return gather(res.results), res


def kernel(x, weight, bias):
    out, _ = run(x, weight, bias)
    return out


# revision 26
# speedup vs baseline: 1.2042x; 1.1070x over previous
"""BinLinear Trainium2 kernel: out = x @ sign(W)^T + sign(bias).

Full shapes: x [8192, 4096] f32, W [4096, 4096] f32, bias [4096] f32,
out [8192, 4096] f32.

Strategy (8 NeuronCores, data-parallel on the token dim M):
  - Each core gets x_shard = x[1024*i : 1024*(i+1)], full W, full bias and
    computes its out shard [1024, 4096]. No collectives; host concatenates.

  - The swizzle-DMA trigger cost on the issuing engine scales with its
    descriptor-run count (~0.47ns per 128B run + ~0.4us fixed), so W's
    64MB of 128B runs is split across TWO HWDGE issuers: W tiles
    alternate between ScalarE (ACT) and SP. ACT-issued tiles use the
    v1-proven claim+hop discipline; SP cannot claim (no data-dep-capable
    cheap op), so SP-issued DMAs rely on the post-scheduling wait
    COMPRESSOR, which walks the scheduled order reconstructing per-sem
    producer-snapshot clocks and drops any wait transitively implied by a
    kept wait (happens-before soundness; build fails if an instruction
    cannot be reduced to walrus's single sync-wait slot).

  - x^T resident as bf16: even k-tiles stream through a gpsimd SWDGE
    cast-load slot CHAIN (fresh DMA targets, zero recycling waits; DVE
    32x32 stream-transpose writes the resident slot); odd k-tiles load as
    f32 on SP through the shared staging pool, DVE-transpose, and
    downcast-copy. Splitting x across two DGE paths roughly halves the
    x-phase critical path that throttled strip 0/1.

  - W streams once as PAIR tiles [L*512 n, 128 k] spanning <=2 n-strips,
    STAGGERED by kt parity so every strip re-produces only half the tiles
    (uniform ~8MB/strip DMA demand). Per tile: 4 swizzled f32 DMAs ->
    3 DVE touches -> DVE bit-trick sign reading the f32 staging's u16
    HIGH halves ((h & 0x8000) | 0x3f80 == +-1.0 bf16 exactly, half the
    input bytes of an f32 ALU op; it carries the 4th DMA-lane wait) ->
    DVE bf16 transpose into kt's resident W^T slot (bufs=1; slot-WAR
    [PE] wait on the old tile's last matmul).

  - TensorE accumulates psum[mi] (8 banks) over 32 k-tiles; rhs is a
    512-col slice of the pair tile. sign(bias) (+-1, same u16 trick)
    enters via a rank-1 ones matmul. PSUM holds the exact output;
    eviction is a plain full-bank DVE copy and the out-DMA (gpsimd)
    follows with its natural RAW wait (own-lane wait compressed away
    through the eviction's slot-WAR chain, outp bufs=8 pinning each slot
    to one DMASW lane).
"""

import numpy as np

import concourse.bass as bass
import concourse.mybir as mybir
import concourse.tile as tile
from concourse.vector_clock import ScopedClock, VectorClock
from concourse.tile import add_dep_helper
from concourse.bass_utils import run_bass_kernel_spmd


class SplitDrainTileContext(tile.TileContext):
    """TileContext whose kernel-tail drain is split into several drain
    instructions. The stock tail emits ONE drain waiting on every active proc
    (engines + all DMA lanes, ~15 waits) which overflows the CTRL
    instruction's sync-wait slots in walrus codegen. Emitting the same waits
    across several drains (1 wait each) is semantically identical."""

    MAX_DRAIN_WAITS = 1

    def _drain_and_barrier(self, tick_clock, wait_clock):
        gc = tick_clock.global_clock
        n = len(gc)
        for lo in range(0, n, self.MAX_DRAIN_WAITS):
            vc = VectorClock()
            for p in range(lo, min(lo + self.MAX_DRAIN_WAITS, n)):
                if gc[p]:
                    vc.require_at_least(p, gc[p])
            drain_inst = self.nc.sync.drain()
            wait_clock.add_sem_waits(
                drain_inst.ins, ScopedClock({None: vc})
            )
        self.nc.all_engine_barrier()
        assert self.sems is not None
        popped = self.nc._tile_sem_poison_stack.pop()
        assert popped is self._sem_poison
        self.nc.clear_and_free_semaphores(list(self.sems.allocated().values()))
        self.nc.all_engine_barrier()


P = 128
NFREE = 512  # moving free dim per matmul (one PSUM bank of fp32)

M_FULL, K_FULL, N_FULL = 8192, 4096, 4096
N_CORES = 8
M_SHARD = M_FULL // N_CORES


def _swizzled_load(nc, sbuf_tile, dram_ap, eng):
    """Load dram_ap ([R, 128] slice) into sbuf_tile [128, R] block-swizzled so
    that a DVE 32x32 stream-transpose of sbuf_tile yields dram_ap.T.

    Pre-DVE we need:  sbuf[32g+a, 32b+c] = dram[32b+a, 32g+c]
    so post-DVE:      out[32g+a, 32b+c] = dram[32b+c, 32g+a] = dram.T[p, f].

    DMA access patterns are limited to 3 dims, so issue one DMA per
    partition-group g (source dims [a, b, c], 128-byte contiguous runs).
    A dtype mismatch (f32 dram -> bf16 sbuf) is legal only on the SWDGE
    (gpsimd) path, which casts during the DMA.
    """
    for g in range(4):
        eng.dma_start(
            sbuf_tile[32 * g : 32 * (g + 1), :],
            dram_ap[:, 32 * g : 32 * (g + 1)].rearrange("(b a) c -> a b c", a=32),
        )


def _touch4(nc, sbuf_tile):
    """In-place 1-element DVE copies, one per partition group. Each waits on
    one of the 4 swizzle DMAs, advancing the DVE's observed semaphore ticks so
    the full-width DVE consumer that follows needs no waits of its own."""
    for g in range(4):
        s = sbuf_tile[32 * g : 32 * (g + 1), 0:1]
        nc.vector.tensor_copy(out=s, in_=s)


# W tile coverage, staggered by kt parity: list of (first_strip, n_strips).
_W_SPANS_EVEN = [(0, 2), (2, 2), (4, 2), (6, 2)]
_W_SPANS_ODD = [(0, 1), (1, 2), (3, 2), (5, 2), (7, 1)]


def _w_spans(kt):
    return _W_SPANS_EVEN if kt % 2 == 0 else _W_SPANS_ODD


def bin_linear_tile_kernel(tc, x_ap, w_ap, b_ap, o_ap):
    nc = tc.nc
    f32 = mybir.dt.float32
    bf16 = mybir.dt.bfloat16
    u16 = mybir.dt.uint16

    MS, K = x_ap.shape  # m per core, contraction
    N = w_ap.shape[0]
    KT = K // P  # k tiles
    KTH = KT // 2
    MT = MS // P  # m tiles (psum banks used per n-strip)
    NS = N // NFREE  # n strips
    assert MT <= 8, "psum accumulators exceed the 8 PSUM banks"
    assert NS == 8 and KT % 2 == 0, "stagger tables assume 8 strips, even KT"

    with (
        tc.tile_pool(name="xe", bufs=1) as xe_pool,
        tc.tile_pool(name="wstg", bufs=4) as wstg_pool,
        tc.tile_pool(name="wsgn", bufs=2) as wsgn_pool,
        tc.tile_pool(name="wt", bufs=1) as wt_pool,
        tc.tile_pool(name="outp", bufs=8) as out_pool,
        tc.tile_pool(name="bias", bufs=1) as bias_pool,
        tc.tile_pool(name="psum", bufs=8, space="PSUM") as psum_pool,
    ):
        # ---- bias + ones first so strip-0's bias matmuls head the queues.
        # half-sign(bias) [1, N] bf16 (+-0.5 exact): SWDGE cast-load
        # f32->bf16, then one in-place DVE bitwise op; ones_row = 2.0 makes
        # the rank-1 bias matmul contribute sign(b) exactly.
        bias_sgn = bias_pool.tile([1, N], bf16)
        ones_row = bias_pool.tile([1, P], bf16)
        pscr = bias_pool.tile([1, 1], f32, name="pscr", tag="pscr", bufs=2)
        ascr = bias_pool.tile([1, 1], f32, name="ascr", tag="ascr", bufs=2)
        nc.gpsimd.dma_start(bias_sgn[:], b_ap[None, :])
        bsu = bias_sgn[:].bitcast(u16)
        nc.vector.tensor_scalar(
            out=bsu,
            in0=bsu,
            scalar1=0x8000,
            scalar2=0x3F00,
            op0=mybir.AluOpType.bitwise_and,
            op1=mybir.AluOpType.bitwise_or,
        )
        nc.scalar.activation(
            ones_row[:],
            bias_sgn[:, 0:P],
            mybir.ActivationFunctionType.Copy,
            bias=2.0,
            scale=0.0,
        )

        # W staging (ACT, bufs=4) uses the v1-proven 1-elem ACT claim +
        # DVE hop, reading the W^T retire beacon from 4 tiles back.
        W_STG_BUFS = 4
        wt_beacons = []

        # ---- x^T resident (gpsimd cast-load slot chain) ---------------
        # One [128, (KT+1)*MS] bf16 tile; the cast DMA for k-tile kt fills
        # chain slot kt+1, the DVE 32x32 stream-transpose writes slot kt =
        # the resident x^T tile. Fresh DMA targets => the casts carry only
        # their own DMASW-lane wait; the LDWEIGHTS' inherited staging-lane
        # waits are removed by the compression pass.
        xe = xe_pool.tile([P, (KT + 1) * MS], bf16, name="xe")

        def xt_slice(kt, lo, hi):
            return xe[:, kt * MS + lo : kt * MS + hi]

        for kt in range(KT):
            stg = xe[:, (kt + 1) * MS : (kt + 2) * MS]
            _swizzled_load(nc, stg, x_ap[:, kt * P : (kt + 1) * P],
                           nc.gpsimd)
            _touch4(nc, stg)
            dst = xe[:, kt * MS : (kt + 1) * MS]
            nc.vector.transpose(dst, stg)
            # in-place x2 (the W path stores +-0.5: 2x * +-0.5 == x*sign)
            nc.vector.tensor_scalar(
                out=dst,
                in0=dst,
                scalar1=2.0,
                scalar2=None,
                op0=mybir.AluOpType.mult,
            )

        # ---- W tile pipeline ------------------------------------------

        def emit_w_tile(kt, span):
            s0, L = span
            R = L * NFREE
            j = len(wt_beacons)
            if j >= W_STG_BUFS:
                h = ascr[0:1, 0:1]
                nc.scalar.activation(
                    h, wt_beacons[j - W_STG_BUFS],
                    mybir.ActivationFunctionType.Copy,
                )
                nc.vector.tensor_copy(out=h, in_=h)
            ws = wstg_pool.tile([P, R], f32, name=f"ws_{kt}_{s0}", tag="ws",
                                bufs=W_STG_BUFS)
            n0 = s0 * NFREE
            _swizzled_load(nc, ws, w_ap[n0 : n0 + R, kt * P : (kt + 1) * P],
                           nc.scalar)
            _touch4(nc, ws)
            wg = wsgn_pool.tile([P, R], bf16, name=f"wg_{kt}_{s0}", tag="wg",
                                bufs=2)
            # half-sign: (w >= 0) - 0.5 in {+0.5, -0.5}, exact in bf16 (the
            # x2 lives in the resident 2*x); all ws readers are DVE so this
            # needs no cross-proc waits after the touches.
            nc.vector.tensor_scalar(
                out=wg[:],
                in0=ws[:],
                scalar1=0.0,
                scalar2=0.5,
                op0=mybir.AluOpType.is_ge,
                op1=mybir.AluOpType.subtract,
            )
            wt = wt_pool.tile([P, R], bf16, name=f"wt_{kt}_{s0}",
                              tag=f"wt{kt}", bufs=1)
            nc.vector.transpose(wt[:], wg[:])
            wt_beacons.append(wt[0:1, 0:1])
            return (wt, s0)

        wt_cur = [emit_w_tile(kt, _w_spans(kt)[0]) for kt in range(KT)]

        def next_span(kt, ns):
            for sp in _w_spans(kt):
                if sp[0] == ns + 1:
                    return sp
            return None

        # ---- PSUM accumulators allocated ONCE (no per-strip realloc waits).
        psums = [
            psum_pool.tile([P, NFREE], f32, name=f"psum_{mi}", tag="acc")
            for mi in range(MT)
        ]

        for ns in range(NS):
            n_lo = ns * NFREE
            # bias enters PSUM first: rank-1 matmul, start=True clears banks.
            for mi in range(MT):
                nc.tensor.matmul(
                    psums[mi][:],
                    ones_row[:],
                    bias_sgn[:, n_lo : n_lo + NFREE],
                    start=True,
                    stop=False,
                )
            for kt in range(KT):
                wt, s0 = wt_cur[kt]
                half = (ns - s0) * NFREE
                rhs = wt[:, half : half + NFREE]
                last = kt == KT - 1
                for mi in range(MT):
                    nc.tensor.matmul(
                        psums[mi][:],
                        xt_slice(kt, mi * P, (mi + 1) * P),
                        rhs,
                        start=False,
                        stop=last,
                    )
                # Refill kt's W slot for the span starting at strip ns+1
                # (emitted after this kt's matmuls so the slot-WAR transpose
                # gates mid-strip; DMA demand is ~16 tiles every strip).
                sp = next_span(kt, ns)
                if sp is not None:
                    wt_cur[kt] = emit_w_tile(kt, sp)

            # A read-only 1-elem DVE copy of the LAST bank: it waits for the
            # final stop-matmul of the strip, putting PE on DVE's clock so
            # every eviction copy below elides its PE wait. (Read-only so
            # the mi=7 evict has no in-pipeline WAR against it.)
            pe_touch = nc.vector.tensor_copy(
                out=pscr[0:1, 0:1], in_=psums[MT - 1][0:1, 0:1]
            )
            # Evict full banks with plain DVE copies (PSUM already holds the
            # exact output); each out-DMA follows with its natural RAW wait
            # (the compressor drops the own-lane wait: bufs=8 pins each out
            # slot to one DMASW lane, so the eviction's slot-WAR wait on the
            # previous strip's out-DMA covers exactly that lane).
            for mi in range(MT):
                ot = out_pool.tile(
                    [P, NFREE], f32, name=f"ot_{ns}_{mi}", tag="ot", bufs=8
                )
                cpi = nc.vector.tensor_copy(out=ot[:], in_=psums[mi][:])
                add_dep_helper(cpi.ins, pe_touch.ins, sync=False,
                               reason="evac copy after PE-observing touch")
                nc.gpsimd.dma_start(
                    o_ap[mi * P : (mi + 1) * P, n_lo : n_lo + NFREE],
                    ot[:],
                )


# Engines whose own-proc-sem waits at past positions are droppable: they are
# single-threaded and retire data effects in queue order. (Pool = 8 Q7 cores
# running concurrently; PE reorders LDWEIGHTS: keep theirs.)
_OWN_DROP_ENGINES = {
    "EngineType.DVE": "DVE",
    "EngineType.Activation": "Activation",
    "EngineType.SP": "SP",
}


def _compress_waits(ordered_insts):
    """Post-scheduling wait compression: drop waits that are transitively
    implied (happens-before) by another wait on the same instruction, so
    every instruction fits walrus's one-sync-wait slot.

    Walking the scheduled order we maintain, per issuing engine, the
    observed clock (join of all waits executed so far plus the producer
    snapshots those waits import), and record for every semaphore update
    (sem, cumulative-value) the producer's knowledge at that point. A wait
    w on instruction X is droppable iff some kept wait (S >= v) on X has
    snapshot(S, v)[w.sem] >= w.value: the kept wait then transitively
    enforces w before X runs, and queue order preserves it for every later
    instruction whose emitted waits assumed X's. Raises if an instruction
    cannot be reduced to a single wait."""
    snap = {}       # (sem id, value) -> dict(sem id -> value), across passes

    def merge(dst, src):
        for s2, v2 in src.items():
            if dst.get(s2, -1) < v2:
                dst[s2] = v2

    def walk(compress):
        # The block list interleaves engines in a non-temporal order; only
        # per-engine subsequences are queue-ordered. Snapshots from earlier
        # passes resolve forward references, converging monotonically.
        cum = {}    # sem id -> cumulative value
        obs = {}    # engine -> dict(sem id -> value)
        failures = []
        for ins in ordered_insts:
            si = ins.sync_info
            if si is None:
                continue
            know = obs.setdefault(ins.engine, {})
            waits = list(si.on_wait)
            know_before = dict(know)
            if waits:
                for w in waits:
                    if w.wait_value is None:
                        continue
                    merge(know, {w.id: w.wait_value})
                    merge(know, snap.get((w.id, w.wait_value), {}))
                if compress and len(waits) > 1:
                    # a wait already implied by the engine's reconstructed
                    # observed clock (its own earlier waits + their
                    # snapshots) is redundant outright; so is a wait on the
                    # engine's OWN proc sem at a past position (in-order
                    # single-threaded engines -- DVE/ACT/SP -- retire
                    # effects in queue order; such waits appear only as
                    # bookkeeping artifacts after TensorScalarPtr ops,
                    # which skip Tile's same-engine elision). Pool is
                    # genuinely multi-core and PE reorders LDWEIGHTS, so
                    # their own-sem waits are kept.
                    own = _OWN_DROP_ENGINES.get(str(ins.engine))
                    needed = [
                        w for w in waits
                        if w.wait_value is not None
                        and know_before.get(w.id, -1) < w.wait_value
                        and not (
                            own is not None
                            and (w.ant_name or "").startswith(own)
                            and cum.get(w.id, 0) >= w.wait_value
                        )
                    ]
                    keeper = needed[0] if len(needed) == 1 else None
                    if keeper is None and len(needed) > 1:
                        for w in needed:
                            ks = dict(snap.get((w.id, w.wait_value), {}))
                            merge(ks, {w.id: w.wait_value})
                            if all(
                                w2 is w
                                or ks.get(w2.id, -1) >= w2.wait_value
                                for w2 in needed
                            ):
                                keeper = w
                                break
                        if keeper is None:
                            failures.append(
                                (ins.name, type(ins).__name__,
                                 str(ins.engine),
                                 [(w.ant_name, w.wait_value) for w in waits])
                            )
                    if keeper is not None:
                        si.on_wait[:] = [keeper]
                    elif not needed:
                        si.on_wait[:] = waits[:1]
            for u in si.on_update:
                if u.update_mode == "sem-inc":
                    inc = 1 if u.update_value is None else u.update_value
                elif u.update_mode == "sem-add-imm" and u.update_value is not None:
                    inc = u.update_value
                else:
                    cum.pop(u.id, None)
                    continue
                nv = cum.get(u.id, 0) + inc
                cum[u.id] = nv
                merged = snap.setdefault((u.id, nv), {})
                merge(merged, know)
        return failures

    walk(False)
    walk(False)
    failures = walk(True)
    if failures:
        raise RuntimeError(
            f"wait compression failed for {len(failures)} instructions: "
            + "; ".join(str(f) for f in failures[:8])
        )


def build_module(m_shard=M_SHARD, k=K_FULL, n=N_FULL):
    nc = bass.Bass("TRN2", target_bir_lowering=False, debug=False)
    f32 = mybir.dt.float32
    x_d = nc.dram_tensor("x", [m_shard, k], f32, kind="ExternalInput")
    w_d = nc.dram_tensor("weight", [n, k], f32, kind="ExternalInput")
    b_d = nc.dram_tensor("bias", [n], f32, kind="ExternalInput")
    o_d = nc.dram_tensor("out", [m_shard, n], f32, kind="ExternalOutput")
    with SplitDrainTileContext(nc) as tc:
        bin_linear_tile_kernel(tc, x_d.ap(), w_d.ap(), b_d.ap(), o_d.ap())
    ordered = []
    for bb, insts in tc.ordered_instructions_by_block.items():
        ordered.extend(insts)
    _compress_waits(ordered)
    return nc


_NC_CACHE = {}


def _get_module():
    if "nc" not in _NC_CACHE:
        _NC_CACHE["nc"] = build_module()
    return _NC_CACHE["nc"]


def make_in_maps(x, weight, bias):
    x = np.ascontiguousarray(np.asarray(x, dtype=np.float32))
    weight = np.ascontiguousarray(np.asarray(weight, dtype=np.float32))
    bias = np.ascontiguousarray(np.asarray(bias, dtype=np.float32))
    return [
        {
            "x": x[i * M_SHARD : (i + 1) * M_SHARD],
            "weight": weight,
            "bias": bias,
        }
        for i in range(N_CORES)
    ]


def gather(results):
    return np.concatenate([results[i]["out"] for i in range(N_CORES)], axis=0)


def run(x, weight, bias, trace=False, **kw):
    """Run on the 8 NeuronCores; returns (out_full, BassKernelResults)."""
    nc = _get_module()
    in_maps = make_in_maps(x, weight, bias)
    res = run_bass_kernel_spmd(nc, in_maps, list(range(N_CORES)), trace=trace, **kw)
    return gather(res.results), res


def kernel(x, weight, bias):
    out, _ = run(x, weight, bias)
    return out
